# revision 2
# baseline (speedup 1.0000x reference)
"""Trainium2 Bass kernel for nn_NonLinearReadoutLayer (equivariant gated MLP readout).

Reference computation (per node, N=200000):
    s = x[:, :128]; v = x[:, 128:].reshape(N, 128, 3)
    h_s = (s @ w1_s) / sqrt(128)                # [N, 256]
    h_v = einsum('nmc,mk->nkc', v, w1_v) / sqrt(128)
    act = silu(h_s[:, :128]); gates = sigmoid(h_s[:, 128:])
    out_s = (act @ w2_s) / sqrt(128)            # [N, 16]
    out_v = einsum('nmc,mk->nkc', h_v * gates[:,:,None], w2_v) / sqrt(128)
    out = concat([out_s, out_v.reshape(N, 48)], 1)   # [N, 64]

Strategy: pure data-parallel over nodes across 8 cores. Host-side marshalling
puts x in feature-major layout xt[f, n] with the vector part de-interleaved
(f = 128 + 128*c + m), so every on-chip op is a clean [128]-contraction matmul
with nodes on the moving/free axis. 1/sqrt(128) is folded into the weights.
The second-layer weights are zero-padded into [128, 64] blocks whose column
index is the final output row, so all four second-layer matmuls accumulate
into one [64, 512] PSUM tile that is the output tile, transposed. The host
un-transposes at the end.

Matmuls run in float32r (TF32-like, ~2e-4 scale-relative err, 4x the fp32
rate). The gate sigmoid is computed as 0.5*(tanh(x/2)+1) because Tanh shares
the ScalarE LUT table set with Silu and Copy ('silu_and_others') — using
Sigmoid directly would force two 1.3us LUT reloads per 512-node tile. The
0.5 factor is folded into the layer-2 vector weights; the +1 is one DVE
tensor_scalar op.
"""

import numpy as np

import concourse.mybir as mybir
import concourse.tile as tile
from concourse import bacc
from concourse.bass_utils import run_bass_kernel_spmd

N_CORES = 8
P = 128
ST = 512  # nodes per matmul group (one PSUM bank of fp32)
MT = 2048  # nodes per DMA megatile
N_TOTAL = 200000
NC_NODES = N_TOTAL // N_CORES  # 25000
NP = 25088  # padded per-core nodes = 49 supertiles

AF = mybir.ActivationFunctionType

_CACHE = {}


def emit_body(nc, pools, xt_ap, out_ap, w):
    """One full pass over the node range. w is the preloaded weight tile."""
    f32 = mybir.dt.float32
    f32r = mybir.dt.float32r
    inp, mid, osbp, ps1, ps2, pso = pools

    w1sa = w[:, 0:128]
    w1sb = w[:, 128:256]
    w1v = w[:, 256:384]
    w2 = [w[:, 384 + 64 * i : 448 + 64 * i] for i in range(4)]

    xt_r = xt_ap.rearrange("(b p) n -> p b n", p=P)

    for m0 in range(0, NP, MT):
        mt = min(MT, NP - m0)
        xin = inp.tile([P, 4, mt], f32r, tag="xin")
        nc.sync.dma_start(out=xin[:], in_=xt_r[:, :, m0 : m0 + mt])
        osb = osbp.tile([64, mt], f32, tag="osb")

        for s0 in range(0, mt, ST):
            sl = slice(s0, s0 + ST)
            # --- layer 1 matmuls ---
            h_sa = ps1.tile([P, ST], f32, tag="hsa")
            nc.tensor.matmul(h_sa[:], w1sa, xin[:, 0, sl], start=True, stop=True)
            h_sg = ps1.tile([P, ST], f32, tag="hsg")
            nc.tensor.matmul(h_sg[:], w1sb, xin[:, 0, sl], start=True, stop=True)
            h_v = []
            for c in range(3):
                t = ps2.tile([P, ST], f32, tag="hv")
                nc.tensor.matmul(t[:], w1v, xin[:, c + 1, sl], start=True, stop=True)
                h_v.append(t)
            # --- gate: sigmoid(x) = 0.5*(tanh(x/2)+1); 0.5 folded into w2v ---
            act = mid.tile([P, ST], f32r, tag="act")
            nc.scalar.activation(act[:], h_sa[:], AF.Silu)
            th = mid.tile([P, ST], f32, tag="th")
            nc.scalar.activation(th[:], h_sg[:], AF.Tanh, scale=0.5)
            t1 = mid.tile([P, ST], f32r, tag="t1")
            nc.vector.tensor_scalar_add(t1[:], th[:], 1.0)
            # --- layer 2: accumulate all irreps into one [64, ST] psum ---
            po = pso.tile([64, ST], f32, tag="po")
            nc.tensor.matmul(po[:], w2[0], act[:], start=True, stop=False)
            for c in range(3):
                gv = mid.tile([P, ST], f32r, tag=f"gv{c}")
                nc.vector.tensor_mul(gv[:], h_v[c][:], t1[:])
                nc.tensor.matmul(po[:], w2[c + 1], gv[:], start=False, stop=(c == 2))
            # --- evacuate to output staging ---
            nc.scalar.copy(osb[:, sl], po[:])

        nc.scalar.dma_start(out=out_ap[0:64, m0 : m0 + mt], in_=osb[:])


def build_nc(reps=1):
    f32 = mybir.dt.float32
    f32r = mybir.dt.float32r
    nc = bacc.Bacc("TRN2", target_bir_lowering=False, debug=False, num_devices=N_CORES)

    xt = nc.dram_tensor("xt", [4 * P, NP], f32r, kind="ExternalInput")
    wcat = nc.dram_tensor("wcat", [P, 640], f32r, kind="ExternalInput")
    outt = nc.dram_tensor("outt", [64, NP], f32, kind="ExternalOutput")

    with tile.TileContext(nc) as tc:
        with (
            tc.tile_pool(name="wsb", bufs=1) as wsb,
            tc.tile_pool(name="inp", bufs=3) as inp,
            tc.tile_pool(name="mid", bufs=2) as mid,
            tc.tile_pool(name="osb", bufs=3) as osbp,
            tc.tile_pool(name="ps1", bufs=1, space="PSUM") as ps1,
            tc.tile_pool(name="ps2", bufs=3, space="PSUM") as ps2,
            tc.tile_pool(name="pso", bufs=3, space="PSUM") as pso,
        ):
            w = wsb.tile([P, 640], f32r)
            nc.sync.dma_start(out=w[:], in_=wcat.ap())
            pools = (inp, mid, osbp, ps1, ps2, pso)
            if reps == 1:
                emit_body(nc, pools, xt.ap(), outt.ap(), w)
            else:
                # reps>1 exists only for the timing harness (bench.py):
                # repeat the whole kernel in a hardware loop so device time
                # dominates the ~100ms axon RPC dispatch overhead.
                with tc.For_i(0, reps, 1):
                    emit_body(nc, pools, xt.ap(), outt.ap(), w)

    nc.compile()
    return nc


# Row permutation: xt row f <- x column perm[f] (de-interleave vector irreps).
def _make_perm():
    perm = np.empty(512, np.int64)
    perm[:128] = np.arange(128)
    m = np.arange(128)
    for c in range(3):
        perm[128 + 128 * c + m] = 128 + 3 * m + c
    return perm


def _prep_weights(w1_s, w1_v, w2_s, w2_v):
    inv = np.float32(1.0 / np.sqrt(128.0))
    w2blk = np.zeros((128, 4, 64), np.float32)
    w2blk[:, 0, 0:16] = w2_s * inv
    for c in range(3):
        # 0.5 from the tanh-form sigmoid: gates = 0.5*(tanh(h/2)+1)
        w2blk[:, c + 1, 16 + 16 * c : 32 + 16 * c] = w2_v * (inv * np.float32(0.5))
    return np.ascontiguousarray(
        np.concatenate([w1_s * inv, w1_v * inv, w2blk.reshape(128, 256)], axis=1)
    )


def _prep_x_core(x, core, perm):
    lo = core * NC_NODES
    xt = np.zeros((512, NP), np.float32)
    xt[:, :NC_NODES] = x[lo : lo + NC_NODES, perm].T
    return xt


def prep_in_maps(x, w1_s, w1_v, w2_s, w2_v):
    wcat = _prep_weights(w1_s, w1_v, w2_s, w2_v)
    perm = _make_perm()
    return [{"xt": _prep_x_core(x, core, perm), "wcat": wcat} for core in range(N_CORES)]


def kernel(x, w1_s, w1_v, w2_s, w2_v):
    x = np.asarray(x, dtype=np.float32)
    wcat = _prep_weights(
        np.asarray(w1_s, np.float32),
        np.asarray(w1_v, np.float32),
        np.asarray(w2_s, np.float32),
        np.asarray(w2_v, np.float32),
    )
    perm = _make_perm()

    if "nc" not in _CACHE:
        _CACHE["nc"] = build_nc()
    nc = _CACHE["nc"]

    in_maps = [
        {"xt": _prep_x_core(x, core, perm), "wcat": wcat} for core in range(N_CORES)
    ]
    res = run_bass_kernel_spmd(nc, in_maps, core_ids=list(range(N_CORES)))

    out = np.empty((N_TOTAL, 64), np.float32)
    for core in range(N_CORES):
        lo = core * NC_NODES
        hi = lo + NC_NODES
        ot = res.results[core]["outt"][:, :NC_NODES]
        out[lo:hi, :16] = ot[:16].T
        out[lo:hi, 16:] = (
            ot[16:].reshape(3, 16, NC_NODES).transpose(2, 1, 0).reshape(NC_NODES, 48)
        )
    return out

